# revision 1
# baseline (speedup 1.0000x reference)
"""Capsule-routing kernel for Trainium2 (8 NeuronCores, data-parallel over batch).

Reference computation (per batch item, S=512 input capsules, N=32 output
capsules of dim D=64, 3 routing iterations):
    u_hat = (u @ W).reshape(S, N, D).transpose(N, S, D)
    b = 0
    for it in 0..2:
        c = softmax(b, axis=capsules)
        o = squash(einsum('ns,nsd->nd', c, u_hat))     # squash = L2 normalize
        if it < 2: b = einsum('nd,nsd->ns', o, u_hat)

Key re-association -- u_hat (S x N*D per batch item) is never materialized.
With m[n,i] = sum_s c[n,s] u[s,i] (one 32x512 matmul per batch item):
    o_pre[n,d] = sum_i m[n,i] W[i, n*64+d]      (block-diag slices of m @ W)
    P[i,n]     = sum_d W[i, n*64+d] o[n,d]      (via W^T chunks, block-diag o)
    b[n,s]     = sum_i u[s,i] P[i,n]
This cuts per-core tensor-engine work ~14x vs. materializing u_hat.
All matmuls run as float32r (full-rate fp32 path); fp32r matmuls require
dst base partition 0, which shapes all the PSUM layouts below.

Distribution: batch 64 -> 8 per core, W replicated, no collectives.
On-core: 2 passes of 4 batch items (u + uT for 8 would overflow SBUF).
uT is produced by the DMA transpose engine straight from DRAM.

Capsule placement (o tiles): n = 4*g + h, g = 2*v + gg:
  o_sb[v][16*h + b, 256*gg + 64*h + d] = o[b, n, d]   (v: tile 0..3, 64 rows)
Pair placement (P-step): pair j = (2j, 2j+1) lives in WT chunk j;
  bd[j][64*h2 + d, 16*h2 + b] = o[b, 2j+h2, d].
"""

import sys

import numpy as np

if "/opt/trn_rl_repo" not in sys.path:
    sys.path.insert(0, "/opt/trn_rl_repo")

import concourse.bass as bass  # noqa: F401
import concourse.mybir as mybir
import concourse.tile as tile
from concourse import bacc
from concourse.masks import make_identity

# All ACT funcs used here (Exp, Ln, Square, Copy, Identity) live together in
# the natural_log_exp_and_others table; putting it first makes the greedy
# table-load pass pick it everywhere, so the kernel needs a single
# LoadActFuncSet instead of ~1.3us swaps between exp/sqrt tables every
# routing iteration.
_orig_get_tables = bacc.get_activation_tables


def _tables_prefer_nle(arch):
    t = _orig_get_tables(arch)
    pref = "natural_log_exp_and_others"
    if pref not in t:
        return t
    mine = t[pref]
    # Keep canonical order (act_func_set_id indexes act_info.json), but make
    # this kernel's functions resolvable only via the preferred table.
    return {k: (v if k == pref else v - mine) for k, v in t.items()}


bacc.get_activation_tables = _tables_prefer_nle

FP = mybir.dt.float32
FR = mybir.dt.float32r
EPS = 1e-7
B, S, I = 64, 512, 512          # full batch, input capsules, input dim
N, D = 32, 64                   # output capsules, capsule dim
NCORES = 8
BC = B // NCORES                # batch per core = 8
BCP = 4                         # batch per on-core pass
P = 128                         # SBUF partitions
SC = S // P                     # s-chunks = 4
IC = I // P                     # i-chunks = 4
ROUTINGS = 3


def _r(ap):
    return ap.bitcast(FR)


def _bcast(col_ap, nfree):
    """[P,1] column -> [P, nfree] step-0 broadcast AP."""
    return bass.AP(tensor=col_ap.tensor, offset=col_ap.offset,
                   ap=[col_ap.ap[0], [0, nfree]])


def build_kernel(nc):
    """Emit the whole per-core program. DRAM I/O: u [BC,S,I], W [I,N*D], out [BC,N,D]."""
    u_dram = nc.dram_tensor("u", [BC, S, I], FP, kind="ExternalInput").ap()
    w_dram = nc.dram_tensor("W", [I, N * D], FP, kind="ExternalInput").ap()
    o_dram = nc.dram_tensor("out", [BC, N, D], FP, kind="ExternalOutput").ap()

    with tile.TileContext(nc) as tc:
        _body(tc, u_dram, w_dram, o_dram)
    return nc


def _body(tc, u_dram, w_dram, o_dram):
    from contextlib import ExitStack

    nc = tc.nc
    ctx = ExitStack()
    with ctx:
        statics = ctx.enter_context(tc.tile_pool(name="statics", bufs=1))
        stage = ctx.enter_context(tc.tile_pool(name="stage", bufs=3))
        psum = ctx.enter_context(tc.tile_pool(name="psum", bufs=5, space="PSUM"))

        # ---------- static SBUF tensors ----------
        ident_f = statics.tile([P, P], FP)
        make_identity(nc, ident_f)
        ident = statics.tile([P, P], FR)
        nc.vector.tensor_copy(ident, ident_f)
        eps_sb = statics.tile([P, 1], FP)
        nc.vector.memset(eps_sb, EPS)
        zcol = statics.tile([P, 1], FP)
        nc.vector.memset(zcol, 0.0)
        ccol = statics.tile([P, 1], FP)
        nc.vector.memset(ccol, 1.0 / N)

        w_sb = statics.tile([P, IC, N * D], FR)       # W[128*ic+p, nd]
        wt_sb = statics.tile([P, N // 2, I], FR)      # W[i, 128*q+p] at [p, q, i]
        u_sb = statics.tile([P, BCP, SC, I], FR)      # u[b, 128*sc+p, i]
        ut_sb = statics.tile([P, BCP, IC, S], FR)     # u[b, s, 128*ic+p] at [p,b,ic,s]
        ct_sb = statics.tile([P, SC, BCP, N], FR)     # c[b, n, s=128*sc+p]
        rt_sb = statics.tile([P, SC, BCP, N], FP)     # r[b, n, s=128*sc+p]
        mt_sb = statics.tile([P, IC, N, 16], FR)      # m[b, n, i]; b-slot pad 16
        pt_sb = statics.tile([P, IC, I], FR)          # P[b,i,n] at [p, ic, 16n+b]
        o_sb = [statics.tile([64, 2 * 4 * D], FR, name=f"o_sb{t}")
                for t in range(4)]
        ot_sb = statics.tile([P, 16 * 64], FR)        # transposed o blocks
        bd_sb = statics.tile([P, N // 2, 32], FR)     # block-diag lhsT per pair

        # fp32r tiles can't be memset directly; broadcast-copy from fp32 cols.
        nc.vector.tensor_copy(
            mt_sb.rearrange("p a b c -> p (a b c)"), _bcast(zcol, IC * N * 16))
        nc.vector.tensor_copy(
            bd_sb.rearrange("p a b -> p (a b)"), _bcast(zcol, (N // 2) * 32))

        def pe_t(dst_psum_ap, src_ap, k=P):
            nc.tensor.transpose(_r(dst_psum_ap), src_ap, ident[:k, :k])

        # ---------- load W; build WT ----------
        w_r = w_dram.rearrange("(c p) n -> p c n", p=P)
        for ic in range(IC):
            nc.sync.dma_start(out=w_sb[:, ic, :], in_=_r(w_r[:, ic, :]))
        with tc.high_priority(offset=-1000):
            for q in range(N // 2):
                tb = psum.tile([P, I], FP, tag="ps", name="tb_w")
                for ic in range(IC):
                    pe_t(tb[:, ic * P:(ic + 1) * P],
                         w_sb[:, ic, q * P:(q + 1) * P])
                if q % 2 == 0:
                    nc.vector.tensor_copy(wt_sb[:, q, :], tb)
                else:
                    nc.scalar.copy(wt_sb[:, q, :], tb)

        for bp in range(BC // BCP):
            _pass(tc, stage, psum, u_dram, o_dram, bp, ident, eps_sb,
                  zcol, ccol, w_sb, wt_sb, u_sb, ut_sb, ct_sb, rt_sb,
                  mt_sb, pt_sb, o_sb, ot_sb, bd_sb, pe_t)


def _pass(tc, stage, psum, u_dram, o_dram, bp, ident, eps_sb,
          zcol, ccol, w_sb, wt_sb, u_sb, ut_sb, ct_sb, rt_sb,
          mt_sb, pt_sb, o_sb, ot_sb, bd_sb, pe_t):
    nc = tc.nc
    b0 = bp * BCP

    # ---------- load u slice; build uT via PE transposes ----------
    for b in range(BCP):
        for sc in range(SC):
            nc.sync.dma_start(
                out=u_sb[:, b, sc, :],
                in_=_r(u_dram[b0 + b, sc * P:(sc + 1) * P, :]),
            )
    with tc.high_priority(offset=-1000):
        for b in range(BCP):
            for ic in range(IC):
                tb = psum.tile([P, S], FP, tag="ps", name="tb_u")
                for sc in range(SC):
                    pe_t(tb[:, sc * P:(sc + 1) * P],
                         u_sb[:, b, sc, ic * P:(ic + 1) * P])
                if ic % 2 == 0:
                    nc.vector.tensor_copy(ut_sb[:, b, ic, :], tb)
                else:
                    nc.scalar.copy(ut_sb[:, b, ic, :], tb)

    # ---------- routing iterations ----------
    for it in range(ROUTINGS):
        if it == 0:
            nc.vector.tensor_copy(
                ct_sb.rearrange("p a b c -> p (a b c)"),
                _bcast(ccol, SC * BCP * N))
        else:
            # softmax over the capsule axis (last, 32-wide), per (s, b).
            # No max-subtraction: |r| stays O(10) here, well inside exp range.
            for b in range(BCP):
                nc.scalar.activation(
                    ct_sb.rearrange("p sc b n -> p b sc n")[:, b],
                    rt_sb.rearrange("p sc b n -> p b sc n")[:, b],
                    mybir.ActivationFunctionType.Exp)
                ssum = stage.tile([P, SC], FP, tag="ssum", name="ssum", bufs=4)
                nc.vector.reduce_sum(
                    ssum, ct_sb.rearrange("p sc b n -> p b sc n")[:, b],
                    axis=mybir.AxisListType.X)
                rsum = stage.tile([P, SC], FP, tag="rsum", name="rsum", bufs=4)
                nc.vector.reciprocal(rsum, ssum)
                rbc = bass.AP(tensor=rsum.tensor, offset=rsum.offset,
                              ap=[rsum.ap[0], [1, SC], [0, N]])
                nc.vector.tensor_tensor(
                    ct_sb.rearrange("p sc b n -> p b sc n")[:, b],
                    ct_sb.rearrange("p sc b n -> p b sc n")[:, b],
                    rbc, mybir.AluOpType.mult)

        # m-step: m[b][n,i] = sum_s c[b,n,s] u[b,s,i]
        # fp32r needs dst base 0 -> one [32,512] psum tile per batch item,
        # then copy + 4 thin transposes each into a shared [128,512] psum.
        pm = [psum.tile([32, I], FP, tag="pmr", name=f"pm{b}", bufs=3) for b in range(BCP)]
        for b in range(BCP):
            for sc in range(SC):
                nc.tensor.matmul(
                    pm[b][:, :],
                    lhsT=ct_sb[:, sc, b, :],
                    rhs=u_sb[:, b, sc, :],
                    start=(sc == 0),
                    stop=(sc == SC - 1),
                )
        tbm = psum.tile([P, I], FP, tag="ps", name="tbm")
        for b in range(BCP):
            msb = stage.tile([32, I], FR, tag="msb", name="msb", bufs=4)
            if b % 2 == 0:
                nc.vector.tensor_copy(msb, pm[b])
            else:
                nc.scalar.copy(msb, pm[b])
            for ic in range(IC):
                pe_t(tbm[:, 128 * b + 32 * ic:128 * b + 32 * ic + 32],
                     msb[:, ic * P:(ic + 1) * P], k=32)
        nc.vector.tensor_copy(
            mt_sb[:, :, :, 0:BCP],
            tbm.rearrange("p (b ic n) -> p ic n b", b=BCP, ic=IC),
        )

        # o-step: o_pre[b,n,d] = sum_i m[b,n,i] W[i, n*64+d]
        # 4-capsule groups g = 2v+gg; out rows 16h+b (base 0), col half gg.
        o_ps = []
        for v in range(4):
            po = psum.tile([64, 2 * 4 * D], FP, tag="ps", name="po")
            o_ps.append(po)
            for gg in range(2):
                g = 2 * v + gg
                for ic in range(IC):
                    nc.tensor.matmul(
                        po[:, 256 * gg:256 * gg + 256],
                        lhsT=mt_sb[:, ic, 4 * g:4 * g + 4, :],
                        rhs=w_sb[:, ic, 4 * g * D:(4 * g + 4) * D],
                        start=(ic == 0),
                        stop=(ic == IC - 1),
                    )
        # squash: o = o_pre / sqrt(sum_d o_pre^2 + eps); valid d-block per
        # row-group h is col block h (within each 256-col half).
        for v in range(4):
            sq = stage.tile([64, 2 * 4 * D], FP, tag="sq", name="sq")
            nc.scalar.square(sq, o_ps[v])
            ss = stage.tile([64, 8], FP, tag="ss", name="ss")
            nc.vector.reduce_sum(
                ss, sq.rearrange("p (c d) -> p c d", c=8),
                axis=mybir.AxisListType.X,
            )
            # rsqrt via exp(-0.5*ln(ss+eps)): keeps ACT in the exp/ln
            # table set, avoiding 1.3us LoadActFuncSet swaps every iter.
            lg = stage.tile([64, 8], FP, tag="lg", name="lg")
            nc.scalar.activation(lg, ss, mybir.ActivationFunctionType.Ln,
                                 bias=eps_sb[0:64, 0:1])
            rn = stage.tile([64, 8], FP, tag="rn", name="rn")
            nc.scalar.activation(rn, lg, mybir.ActivationFunctionType.Exp,
                                 scale=-0.5)
            rnb = bass.AP(tensor=rn.tensor, offset=rn.offset,
                          ap=[rn.ap[0], [1, 8], [0, D]])
            nc.vector.tensor_tensor(
                o_sb[v].rearrange("p (c d) -> p c d", c=8),
                o_ps[v].rearrange("p (c d) -> p c d", c=8),
                rnb, mybir.AluOpType.mult)

        # oT: transpose 128-col blocks k of each o tile; block (v,k) lands at
        # ot col 64*(4v+k). Valid: rows 64*h2+d, cols 16h+b for h=2(k%2)+h2.
        for w2 in range(2):
            tbo = psum.tile([P, 8 * 64], FP, tag="ps", name="tbo")
            for kk in range(8):
                blk = 8 * w2 + kk
                v, k = blk // 4, blk % 4
                pe_t(tbo[:, 64 * kk:64 * kk + 64],
                     o_sb[v][:, 128 * k:128 * k + 128], k=64)
            nc.vector.tensor_copy(ot_sb[:, w2 * 512:(w2 + 1) * 512], tbo)

        if it == ROUTINGS - 1:
            # compact: co[(hh,d), b, blk] = o[b, n=2*blk+hh, d]; 2 DMAs out.
            co = stage.tile([P, BCP, 16], FR, tag="co", name="co")
            for blk in (0, 1, 4, 5, 8, 9, 12, 13):
                for hh in range(2):
                    h = 2 * (blk % 2) + hh
                    col = 64 * blk + 16 * h
                    s0 = ot_sb[64 * hh:64 * hh + 64, col:col + BCP]
                    src_ap = bass.AP(tensor=s0.tensor, offset=s0.offset,
                                     ap=[s0.ap[0], [128, 2], [1, BCP]])
                    d0 = co[64 * hh:64 * hh + 64, 0:BCP, blk]
                    dst_ap = bass.AP(tensor=d0.tensor, offset=d0.offset,
                                     ap=[d0.ap[0], [2, 2], [16, BCP]])
                    nc.vector.tensor_copy(dst_ap, src_ap)
            for hh in range(2):
                dst = bass.AP(
                    tensor=o_dram.tensor,
                    offset=o_dram.offset + b0 * N * D + hh * D,
                    ap=[[1, D], [N * D, BCP], [2 * D, 16]],
                )
                nc.sync.dma_start(out=dst, in_=co[64 * hh:64 * hh + 64].bitcast(FP))
            continue

        # bd[j][64h2+d, 16h2+b] = o[b, 2j+h2, d]
        for j in (0, 1, 4, 5, 8, 9, 12, 13):
            v = j // 4
            k = 2 * ((j // 2) % 2) + (j % 2)
            for h2 in range(2):
                h = 2 * (j % 2) + h2
                col = 64 * (4 * v + k) + 16 * h
                s0 = ot_sb[64 * h2:64 * h2 + 64, col:col + BCP]
                src_ap = bass.AP(tensor=s0.tensor, offset=s0.offset,
                                 ap=[s0.ap[0], [128, 2], [1, BCP]])
                d0 = bd_sb[64 * h2:64 * h2 + 64, j, 16 * h2:16 * h2 + BCP]
                dst_ap = bass.AP(tensor=d0.tensor, offset=d0.offset,
                                 ap=[d0.ap[0], [64, 2], [1, BCP]])
                nc.vector.tensor_copy(dst_ap, src_ap)

        # P-step (transposed output): for pair j, i-chunk ic:
        #   pt[i, 16n+b] = sum_{(h,d)} WT[j][(h,d), i] * bd[j][(h,d), 16h+b]
        pt_ps = [psum.tile([P, I], FP, tag="ps", name=f"pp{ic}")
                 for ic in range(IC)]
        for ic in range(IC):
            for j in range(N // 2):
                nc.tensor.matmul(
                    pt_ps[ic][:, 32 * j:32 * j + 32],
                    lhsT=wt_sb[:, j, ic * P:(ic + 1) * P],
                    rhs=bd_sb[:, j, :],
                    start=True, stop=True,
                )
            if ic % 2 == 0:
                nc.vector.tensor_copy(pt_sb[:, ic, :], pt_ps[ic])
            else:
                nc.scalar.copy(pt_sb[:, ic, :], pt_ps[ic])

        # r-step: r[b][n,s] = sum_i P[b,i,n] uT[b][i,s]; same dance as m.
        prs = [psum.tile([32, S], FP, tag="pmr", name=f"pr{b}", bufs=3) for b in range(BCP)]
        for b in range(BCP):
            for ic in range(IC):
                nc.tensor.matmul(
                    prs[b][:, :],
                    lhsT=pt_sb[:, ic, :]
                        .rearrange("p (n c) -> p c n", c=16)[:, b, :],
                    rhs=ut_sb[:, b, ic, :],
                    start=(ic == 0),
                    stop=(ic == IC - 1),
                )
        tbr = psum.tile([P, S], FP, tag="ps", name="tbr")
        for b in range(BCP):
            rsb = stage.tile([32, S], FR, tag="rsb", name="rsb", bufs=4)
            if b % 2 == 0:
                nc.vector.tensor_copy(rsb, prs[b])
            else:
                nc.scalar.copy(rsb, prs[b])
            for sc in range(SC):
                pe_t(tbr[:, 128 * b + 32 * sc:128 * b + 32 * sc + 32],
                     rsb[:, sc * P:(sc + 1) * P], k=32)
            nc.vector.tensor_copy(
                rt_sb[:, :, b, :],
                tbr[:, 128 * b:128 * (b + 1)]
                    .rearrange("p (sc n) -> p sc n", sc=SC),
            )


_COMPILED = None


def _get_compiled():
    global _COMPILED
    if _COMPILED is None:
        nc = bacc.Bacc("TRN2", target_bir_lowering=False, debug=False,
                       num_devices=NCORES)
        build_kernel(nc)
        nc.compile()
        _COMPILED = nc
    return _COMPILED


def kernel(u_vecs, W):
    from concourse.bass_utils import run_bass_kernel_spmd

    u_vecs = np.ascontiguousarray(u_vecs, dtype=np.float32)
    W = np.ascontiguousarray(W, dtype=np.float32)
    assert u_vecs.shape == (B, S, I) and W.shape == (I, N * D)

    nc = _get_compiled()
    in_maps = [
        {"u": u_vecs[c * BC:(c + 1) * BC], "W": W} for c in range(NCORES)
    ]
    res = run_bass_kernel_spmd(nc, in_maps, list(range(NCORES)))
    return np.concatenate(
        [res.results[c]["out"] for c in range(NCORES)], axis=0
    ).astype(np.float32)



# revision 10
# speedup vs baseline: 1.9096x; 1.9096x over previous
"""Capsule-routing kernel for Trainium2 (8 NeuronCores, data-parallel over batch).

Reference computation (per batch item, S=512 input capsules, N=32 output
capsules of dim D=64, 3 routing iterations):
    u_hat = (u @ W).reshape(S, N, D).transpose(N, S, D)
    b = 0
    for it in 0..2:
        c = softmax(b, axis=capsules)
        o = squash(einsum('ns,nsd->nd', c, u_hat))     # squash = L2 normalize
        if it < 2: b = einsum('nd,nsd->ns', o, u_hat)

u_hat is never materialized. With m[b][n,i] = sum_s c[n,s] u[s,i]:
    o_pre[n,d] = sum_i m[n,i] W[i, n*64+d]
    P[i,n]     = sum_d W[i, n*64+d] o[n,d]
    r[n,s]     = sum_i u[s,i] P[i,n]
All matmuls run in fp16 (1 PE cycle/row at any output width, where fp32r
pays 4x below 256), with fp32 PSUM accumulation; softmax/squash math stays
fp32.  Each matmul is oriented so its PSUM output lands in the layout its
consumer wants (mT[i,n], oT[d,(n,b)], P[i,(n,b)], rT[s,n]), eliminating
nearly all shuffle copies of the previous design.

Distribution: batch 64 -> 8 per core, W replicated, no collectives.
The 8 items run as two groups of 4 so the squash/normalize steps (which
batch a whole group) don't barrier early items on late-arriving DMA.
DMA order: W chunks, then u per item; compute pipelines behind the
(serialized) DMA stream.

Capsule split by parity: oT holds even capsules on partitions 0:64 and odd
on 64:128 (column (nh,b'), n = 2*nh+par), which makes the block-diag rhs for
the P-step two partition-aligned strided copies and the squash reduction a
single 128-row ones-matmul.
"""

import sys

import numpy as np

if "/opt/trn_rl_repo" not in sys.path:
    sys.path.insert(0, "/opt/trn_rl_repo")

import concourse.bass as bass  # noqa: F401
import concourse.mybir as mybir
import concourse.tile as tile
from concourse import bacc
from concourse.masks import make_identity

# All ACT funcs used here (Exp, Ln, Square, Copy) live together in the
# natural_log_exp_and_others table; putting it first makes the greedy
# table-load pass pick it everywhere, so the kernel needs a single
# LoadActFuncSet instead of ~1.3us swaps between exp/sqrt tables every
# routing iteration.
_orig_get_tables = bacc.get_activation_tables


def _tables_prefer_nle(arch):
    t = _orig_get_tables(arch)
    pref = "natural_log_exp_and_others"
    if pref not in t:
        return t
    mine = t[pref]
    return {k: (v if k == pref else v - mine) for k, v in t.items()}


bacc.get_activation_tables = _tables_prefer_nle

FP = mybir.dt.float32
F16 = mybir.dt.float16
EPS = 1e-7
B, S, I = 64, 512, 512          # full batch, input capsules, input dim
N, D = 32, 64                   # output capsules, capsule dim
NCORES = 8
BC = B // NCORES                # batch per core = 8
P = 128                         # SBUF partitions
SC = S // P                     # s-chunks = 4
IC = I // P                     # i-chunks = 4
NH = N // 2                     # capsule pairs = 16
G = 2                           # batch groups per core
BG = BC // G                    # batch per group = 4
ROUTINGS = 3


def build_kernel(nc):
    """Emit the whole per-core program. DRAM I/O: u [BC,S,I], W [I,N*D], out [BC,N,D]."""
    u_dram = nc.dram_tensor("u", [BC, S, I], FP, kind="ExternalInput").ap()
    w_dram = nc.dram_tensor("W", [I, N * D], FP, kind="ExternalInput").ap()
    o_dram = nc.dram_tensor("out", [BC, N, D], FP, kind="ExternalOutput").ap()

    with tile.TileContext(nc) as tc:
        _body(tc, u_dram, w_dram, o_dram)
    return nc


def _body(tc, u_dram, w_dram, o_dram):
    from contextlib import ExitStack

    nc = tc.nc
    ctx = ExitStack()
    with ctx:
        statics = ctx.enter_context(tc.tile_pool(name="statics", bufs=1))
        stage = ctx.enter_context(tc.tile_pool(name="stage", bufs=1))
        psum = ctx.enter_context(tc.tile_pool(name="psum", bufs=1, space="PSUM"))

        # ---------- static SBUF tensors ----------
        ident_f = statics.tile([P, P], FP)
        make_identity(nc, ident_f)
        ident = statics.tile([P, P], F16)
        nc.vector.tensor_copy(ident, ident_f)
        eps_sb = statics.tile([P, 1], FP)
        nc.vector.memset(eps_sb, EPS)

        # f16 constants built from f32 staging (memset on f16 is dicey)
        c32 = statics.tile([P, 3], FP)
        nc.vector.memset(c32, 0.0)
        nc.vector.memset(c32[:, 0:1], 1.0)                  # ones col
        nc.vector.memset(c32[0:64, 1:2], 1.0)               # parity sel col 0
        nc.vector.memset(c32[64:128, 2:3], 1.0)             # parity sel col 1
        onec = statics.tile([P, 1], F16)
        nc.vector.tensor_copy(onec, c32[:, 0:1])
        ones2 = statics.tile([P, 2], F16)
        nc.vector.tensor_copy(ones2, c32[:, 1:3])
        # sel2T rows 0:2 = ones2^T (row0 = [1]*64+[0]*64, row1 = inverse),
        # built by PE transpose: writing single partitions directly is not a
        # legal engine access.
        sel2T = statics.tile([P, P], F16)

        u16 = statics.tile([P, BC, SC, I], F16)       # u[b, 128*sc+p, i]
        ut16 = statics.tile([P, BC, IC, S], F16)      # u[b, s, 128*ic+p] at [p,b,ic,s]
        w16 = statics.tile([P, IC, N * D], F16)       # W[128*ic+p, nd]
        wt16 = statics.tile([P, NH, I], F16)          # W[i, 64*(2j+h)+d] at [(64h+d), j, i]
        su_sb = statics.tile([P, IC, BC], F16)        # rowsum_s u[b,s,i] at [p, ic, b]
        mt = statics.tile([P, BC, IC, N], F16)        # m[b][n, 128ic+p] at [p,b,ic,n]
        pt = statics.tile([P, G, IC, NH, 2, BG], F16)  # P[b][128ic+p, n=2j+h]
        ct = statics.tile([P, BC, SC, N], F16)        # c[b][n, 128sc+p]
        exps = statics.tile([P, BC, SC * N], FP)      # exp(r) staging
        ot = statics.tile([P, G, NH * BG], F16)       # squashed oT: [(64par+d), g, (nh,b')]
        oc = statics.tile([P, G, NH * BG], F16)       # unsquashed oT (psum->sbuf)
        sqb = statics.tile([P, G, NH * BG], F16)      # oT squares
        bd = statics.tile([P, G, NH, 2 * BG], F16)    # block-diag rhs for P-step
        lg = statics.tile([P, G, NH * BG], FP)        # ln(ss+eps), rows 0:2
        rs = statics.tile([P, G, NH * BG], F16)       # rsqrt, rows 0:2
        co_sb = statics.tile([P, 2 * D], FP)          # output staging

        # zero bd via broadcast copy from f32 zero col
        zc = statics.tile([P, 1], FP)
        nc.vector.memset(zc, 0.0)
        bd_flat = bd.rearrange("p a b c -> p (a b c)")
        nc.vector.tensor_copy(
            bd_flat,
            bass.AP(tensor=zc.tensor, offset=zc.offset,
                    ap=[zc.ap[0], [0, G * NH * 2 * BG]]),
        )

        def tr(dst_psum_ap, src_ap):
            nc.tensor.transpose(dst_psum_ap, src_ap, ident)

        tp0 = psum.tile([P, S], F16, tag="tp", name="tp_sel", bufs=2)
        tr(tp0[0:2, 0:P], ones2)
        nc.vector.tensor_copy(sel2T[0:2, :], tp0[0:2, 0:P])

        # ---------- W: DMA, convert (Pool), build wt via PE transposes ----------
        for ic in range(IC):
            wstage = stage.tile([P, N * D], FP, tag="wstage", name="wstage", bufs=2)
            nc.sync.dma_start(out=wstage, in_=w_dram[ic * P:(ic + 1) * P, :])
            nc.gpsimd.tensor_copy(w16[:, ic, :], wstage)
        for j in range(NH):
            tp = psum.tile([P, I], F16, tag="tp", name="tp_w", bufs=2)
            for ic in range(IC):
                tr(tp[:, ic * P:(ic + 1) * P], w16[:, ic, j * P:(j + 1) * P])
            if j % 2 == 0:
                nc.vector.tensor_copy(wt16[:, j, :], tp)
            else:
                nc.scalar.copy(wt16[:, j, :], tp)

        # osq psum tiles for it0 are allocated up-front: their spare column
        # range doubles as the it0 rowsum (su) accumulator region.
        osq0 = [psum.tile([P, 256], FP, tag="osq", name=f"osq0g{g}", bufs=2)
                for g in range(G)]

        # ---------- u: DMA halves, convert, transpose, it0 rowsums ----------
        for b in range(BC):
            g, bp = b // BG, b % BG
            for h in range(2):
                ust = stage.tile([P, 2, I], FP, tag="ustage", name="ustage", bufs=3)
                nc.sync.dma_start(
                    out=ust,
                    in_=u_dram[b, 256 * h:256 * h + 256, :]
                        .rearrange("(c p) i -> p c i", p=P),
                )
                if h == 0:
                    nc.scalar.copy(u16[:, b, 2 * h:2 * h + 2, :], ust)
                else:
                    nc.gpsimd.tensor_copy(u16[:, b, 2 * h:2 * h + 2, :], ust)
            for ic in range(IC):
                tp = psum.tile([P, S], F16, tag="tp", name="tp_u", bufs=2)
                for sc in range(SC):
                    tr(tp[:, sc * P:(sc + 1) * P],
                       u16[:, b, sc, ic * P:(ic + 1) * P])
                if ic % 2 == 0:
                    nc.vector.tensor_copy(ut16[:, b, ic, :], tp)
                else:
                    nc.scalar.copy(ut16[:, b, ic, :], tp)
            # it0 c is uniform -> m0[i,n] is (scaled) rowsum of u, n-independent;
            # squash is scale-invariant so the 1/N factor is dropped.
            for ic in range(IC):
                for sc in range(SC):
                    nc.tensor.matmul(
                        osq0[g][:, 208 + 4 * bp + ic:208 + 4 * bp + ic + 1],
                        lhsT=u16[:, b, sc, ic * P:(ic + 1) * P],
                        rhs=onec,
                        start=(sc == 0), stop=(sc == SC - 1),
                    )
            if bp == BG - 1:
                src = bass.AP(tensor=osq0[g].tensor,
                              offset=osq0[g].offset + 208,
                              ap=[osq0[g].ap[0], [1, IC], [IC, BG]])
                dst = bass.AP(tensor=su_sb.tensor,
                              offset=su_sb.offset + 4 * g,
                              ap=[su_sb.ap[0], [BC, IC], [1, BG]])
                nc.vector.tensor_copy(dst, src)

        # ---------- routing ----------
        for it in range(ROUTINGS):
            for g in range(G):
                if it > 0:
                    # m-step (transposed): mT[i, n] per item
                    mt_ps = psum.tile([P, 512], FP, tag=f"mrt{g}",
                                      name=f"mt{it}g{g}", bufs=1)
                    for bp in range(BG):
                        b = BG * g + bp
                        for ic in range(IC):
                            for sc in range(SC):
                                nc.tensor.matmul(
                                    mt_ps[:, 128 * bp + 32 * ic:128 * bp + 32 * ic + 32],
                                    lhsT=u16[:, b, sc, ic * P:(ic + 1) * P],
                                    rhs=ct[:, b, sc, :],
                                    start=(sc == 0), stop=(sc == SC - 1),
                                )
                        nc.vector.tensor_copy(
                            mt[:, b, :, :].rearrange("p a b -> p (a b)"),
                            mt_ps[:, 128 * bp:128 * (bp + 1)])

                # o-step: oT[(par,d), (nh,b')] per group; even capsules on
                # partitions 0:64, odd on 64:128.
                osq = osq0[g] if it == 0 else psum.tile(
                    [P, 256], FP, tag="osq", name=f"osq{it}g{g}", bufs=2)
                for n in range(N):
                    nh, par = n >> 1, n & 1
                    for ic in range(IC):
                        if it == 0:
                            rhs = su_sb[:, ic, BG * g:BG * (g + 1)]
                        else:
                            rhs = mt[:, BG * g:BG * (g + 1), ic, n]
                        nc.tensor.matmul(
                            osq[64 * par:64 * par + 64, 4 * nh:4 * nh + 4],
                            lhsT=w16[:, ic, D * n:D * (n + 1)],
                            rhs=rhs,
                            start=(ic == 0), stop=(ic == IC - 1),
                        )

                # squash: o / sqrt(sum_d o^2 + eps); rsqrt via exp(-.5*ln)
                # to stay inside the single ACT table set.  The DVE can read
                # only one PSUM operand, so oT drops to SBUF (oc) in parallel
                # with the rsqrt chain.
                nc.scalar.square(sqb[:, g, :], osq[:, 0:64])
                nc.vector.tensor_copy(oc[:, g, :], osq[:, 0:64])
                nc.tensor.matmul(osq[0:2, 64:128], lhsT=ones2,
                                 rhs=sqb[:, g, :], start=True, stop=True)
                nc.scalar.activation(lg[0:2, g, :], osq[0:2, 64:128],
                                     mybir.ActivationFunctionType.Ln,
                                     bias=eps_sb[0:2, 0:1])
                nc.scalar.activation(rs[0:2, g, :], lg[0:2, g, :],
                                     mybir.ActivationFunctionType.Exp,
                                     scale=-0.5)
                nc.tensor.matmul(osq[:, 128:192], lhsT=sel2T[0:2, :],
                                 rhs=rs[0:2, g, :], start=True, stop=True)
                nc.vector.tensor_tensor(ot[:, g, :], oc[:, g, :],
                                        osq[:, 128:192], mybir.AluOpType.mult)

                if it < ROUTINGS - 1:
                    # block-diag rhs: bd[(64h+d), g, j, 4h+b'] = o[b, 2j+h, d]
                    for par in range(2):
                        src = ot[64 * par:64 * par + 64, g, :] \
                            .rearrange("p (j c) -> p j c", j=NH)
                        dst = bd[64 * par:64 * par + 64, g, :,
                                 4 * par:4 * par + BG]
                        nc.vector.tensor_copy(dst, src)
                    # P-step: P[i, n] for both capsules of pair j at once
                    pt_ps = psum.tile([P, 512], FP, tag=f"pt{g}",
                                      name=f"pt{it}g{g}", bufs=1)
                    for ic in range(IC):
                        for j in range(NH):
                            nc.tensor.matmul(
                                pt_ps[:, 128 * ic + 8 * j:128 * ic + 8 * j + 8],
                                lhsT=wt16[:, j, ic * P:(ic + 1) * P],
                                rhs=bd[:, g, j, :],
                                start=True, stop=True,
                            )
                    # copy halves so the r-step's first accumulation chunk
                    # starts ~300ns earlier
                    pt_flat = pt[:, g].rearrange("p a b c d -> p (a b c d)")
                    nc.vector.tensor_copy(pt_flat[:, 0:256], pt_ps[:, 0:256])
                    nc.vector.tensor_copy(pt_flat[:, 256:512], pt_ps[:, 256:512])

                    # r-step (transposed): rT[s, n] per item, then softmax
                    rt_ps = psum.tile([P, 512], FP, tag=f"mrt{g}",
                                      name=f"rt{it}g{g}", bufs=1)
                    for bp in range(BG):
                        b = BG * g + bp
                        for sc in range(SC):
                            for ic in range(IC):
                                nc.tensor.matmul(
                                    rt_ps[:, 128 * bp + 32 * sc:128 * bp + 32 * sc + 32],
                                    lhsT=ut16[:, b, ic, sc * P:(sc + 1) * P],
                                    rhs=pt[:, g, ic, :, :, bp],
                                    start=(ic == 0), stop=(ic == IC - 1),
                                )
                        # softmax over the capsule axis (32-wide groups).
                        # |r| stays O(10): exp is safe in fp32.
                        nc.scalar.activation(exps[:, b, :],
                                             rt_ps[:, 128 * bp:128 * (bp + 1)],
                                             mybir.ActivationFunctionType.Exp)
                        e3 = exps[:, b, :].rearrange("p (c n) -> p c n", c=SC)
                        ssum = stage.tile([P, SC], FP, tag="ssum", name="ssum", bufs=4)
                        nc.vector.reduce_sum(ssum, e3, axis=mybir.AxisListType.X)
                        rsum = stage.tile([P, SC], FP, tag="rsum", name="rsum", bufs=4)
                        nc.vector.reciprocal(rsum, ssum)
                        rbc = bass.AP(tensor=rsum.tensor, offset=rsum.offset,
                                      ap=[rsum.ap[0], [1, SC], [0, N]])
                        nc.vector.tensor_tensor(
                            ct[:, b, :, :], e3, rbc, mybir.AluOpType.mult)
                else:
                    # output: transpose ot -> [(nh,b'), (par,d)] rows, f32 copy
                    co_ps = osq[:, 192:256].bitcast(F16)
                    nc.tensor.transpose(co_ps[64 * g:64 * g + 64, :],
                                        ot[:, g, :], ident)
                    nc.vector.tensor_copy(co_sb[64 * g:64 * g + 64, :],
                                          co_ps[64 * g:64 * g + 64, :])

        # output DMAs: partition (g, nh, b') -> rows n=2nh,2nh+1 of item 4g+b'
        for g in range(G):
            dst = bass.AP(
                tensor=o_dram.tensor,
                offset=o_dram.offset + g * BG * N * D,
                ap=[[2 * D, NH], [N * D, BG], [1, 2 * D]],
            )
            nc.sync.dma_start(out=dst, in_=co_sb[64 * g:64 * g + 64, :])


_COMPILED = None


def _get_compiled():
    global _COMPILED
    if _COMPILED is None:
        nc = bacc.Bacc("TRN2", target_bir_lowering=False, debug=False,
                       num_devices=NCORES)
        build_kernel(nc)
        nc.compile()
        _COMPILED = nc
    return _COMPILED


def kernel(u_vecs, W):
    from concourse.bass_utils import run_bass_kernel_spmd

    u_vecs = np.ascontiguousarray(u_vecs, dtype=np.float32)
    W = np.ascontiguousarray(W, dtype=np.float32)
    assert u_vecs.shape == (B, S, I) and W.shape == (I, N * D)

    nc = _get_compiled()
    in_maps = [
        {"u": u_vecs[c * BC:(c + 1) * BC], "W": W} for c in range(NCORES)
    ]
    res = run_bass_kernel_spmd(nc, in_maps, list(range(NCORES)))
    return np.concatenate(
        [res.results[c]["out"] for c in range(NCORES)], axis=0
    ).astype(np.float32)
